# revision 1
# baseline (speedup 1.0000x reference)
"""Bass kernel for nn_CausalAttention: B=2, L=2048, C=1024, H=16, hd=64 on 8 cores.

Sharding: 2 heads per core (tensor parallel). Each core computes
qkv for its heads, RoPE, causal attention, and a partial projection
(its 128 channels x full Wproj rows slice) -> [4096, 1024] partial.
Host sums partials.

Layouts (per core, heads h0, h1):
  qT tile [128, 4096]: rows = [h0-even d (32), h0-odd d (32), h1-even, h1-odd]
     (via host permutation of Wq columns), tokens = b*2048 + l.
  kT tile: same. vT tile [128, 4096]: rows = [h0 d(64), h1 d(64)] natural.
  RoPE: Q = T + U where T = qT * TA  (TA = [A;A;A;A] 32-row blocks, A = sin table)
        U[g] = qT[swap(g)] * TB[swap(g)] with 32-row-shifted outputs,
        TB blocks = [+B, -B, +B, -B] (B = cos table).
  v-nat: PE-transpose of vT 128x128 blocks -> [128 tok, 16 kt, 66*2] with a
        ones column at 64 (h0) and 130 (h1) for fused sumexp.
  scoresT psum [128 k, 512 q] = matmul(lhsT=K[base 0 or 64, k-tile], rhs=Q[same base, q-block])
  attT = exp(scoresT) via ACT -> f32r SBUF; diagonal tiles masked by DVE mult.
  AV: psum [65, 512] += matmul(lhsT=v_nat[:, kt, head*66 : head*66+65], rhs=attT)
  normalize: recip(row 64) -> bcast via K=1 matmul -> DVE mult -> att_sb [128, 512]
  proj: psum [128, 512] = matmul(lhsT=att_sb[:, mt*128:...], rhs=Wproj[:, n*512:...])
"""
import math
import numpy as np
from contextlib import ExitStack

import concourse.bass as bass
import concourse.mybir as mybir
import concourse.tile as tile
from concourse.vector_clock import ScopedClock

F32 = mybir.dt.float32
F32R = mybir.dt.float32r
AX = mybir.AluOpType

B, L, C = 2, 2048, 1024
H, HD = 16, 64
T = B * L          # 4096 tokens
NC_CORES = 8
HPC = H // NC_CORES  # heads per core = 2
QB = 512             # q block
KT = 128             # k tile
N_QB = L // QB       # 4 q blocks per batch
N_KT_B = L // KT     # 16 k tiles per batch


# ---------------------------------------------------------------- tile patch
def _patched_drain_and_barrier(self, tick_clock, wait_clock):
    nc = self.nc
    drain_inst = nc.sync.drain()
    wait_clock.add_sem_waits(
        drain_inst.ins, ScopedClock({None: tick_clock.global_clock})
    )
    si = drain_inst.ins.sync_info
    if si is not None and si.on_wait and len(si.on_wait) > 1:
        waits = list(si.on_wait)
        drain_inst.ins.sync_info = mybir.SyncInfo(
            on_wait=waits[:1], on_update=list(si.on_update or [])
        )
        for w in waits[1:]:
            nop = nc.sync.nop(nofuse=True)
            nop.ins.sync_info = mybir.SyncInfo(on_wait=[w], on_update=[])
    nc.all_engine_barrier()
    assert self.sems is not None
    popped = nc._tile_sem_poison_stack.pop()
    assert popped is self._sem_poison
    nc.clear_and_free_semaphores(list(self.sems.allocated().values()))
    nc.all_engine_barrier()


def apply_tile_patch():
    tile.TileContext._drain_and_barrier = _patched_drain_and_barrier


def split_excess_waits(nc, cap=1):
    """Walrus build rejects instructions carrying more than a couple of sync
    waits; move excess waits onto same-engine NoOp carriers inserted right
    before the instruction."""
    # temp bb to swallow builder appends
    for f in nc.m.functions:
        for bb in f.blocks:
            new = []
            for inst in bb.instructions:
                si = inst.sync_info
                waits = list(si.on_wait) if si is not None and si.on_wait else []
                if len(waits) > cap:
                    inst.sync_info = mybir.SyncInfo(
                        on_wait=waits[:cap], on_update=list(si.on_update or []))
                    for w in waits[cap:]:
                        nop = nc.engines[inst.engine].nop(nofuse=True)
                        # remove from wherever the builder appended it
                        cur = nc.cur_bb.bb.instructions
                        assert cur and cur[-1].name == nop.ins.name
                        cur.pop()
                        nop.ins.sync_info = mybir.SyncInfo(on_wait=[w], on_update=[])
                        new.append(nop.ins)
                new.append(inst)
            bb.instructions = new


# ---------------------------------------------------------------- host prep
def host_prep():
    """Core-independent prep: rope tables, masks, identity."""
    pos = np.arange(L, dtype=np.float64)[:, None]
    dim = np.arange(0, HD, 2, dtype=np.float64)
    freq = pos / (10000.0 ** (dim / HD))      # [L, 32]
    A = np.sin(freq).astype(np.float32)       # 'cos' in ref naming
    Bc = np.cos(freq).astype(np.float32)      # 'sin' in ref naming
    AT = np.ascontiguousarray(A.T)            # [32, L]
    BT = np.ascontiguousarray(Bc.T)
    # TA [128, 4096] = [A;A;A;A] blocks, tokens tiled over batches
    TA = np.tile(AT, (4, B))
    TB = np.tile(np.concatenate([BT, -BT], axis=0), (2, B))  # [+B,-B,+B,-B]
    # mask-add matmul factors: scores += W1^T @ W2kt = -BIG * 1[kp > qf - kt*128]
    BIG = 30.0
    W1 = np.zeros((KT, KT), dtype=np.float32)
    jj = np.arange(KT)[:, None]; kp = np.arange(KT)[None, :]
    W1[:127, :] = -BIG * (kp > jj[:127]).astype(np.float32)
    W1[127, :] = -BIG
    W2 = np.zeros((4, KT, QB), dtype=np.float32)
    qf = np.arange(QB)[None, :]
    for kt in range(4):
        r = qf - kt * KT                       # [1, 512]
        for j in range(127):
            W2[kt, j] = (r[0] == j).astype(np.float32)
        W2[kt, 127] = (r[0] < 0).astype(np.float32)
    ident = np.eye(128, dtype=np.float32)
    return TA, TB, W1, W2, ident


def shard_inputs(x, Wqkv, Wproj):
    """Returns per-core input dicts."""
    x2 = np.ascontiguousarray(x.reshape(T, C))
    xT = np.ascontiguousarray(x2.T)                      # [C, T]
    Wq = Wqkv[:, 0 * C:1 * C]
    Wk = Wqkv[:, 1 * C:2 * C]
    Wv = Wqkv[:, 2 * C:3 * C]
    TA, TB, W1, W2, ident = host_prep()
    scale = 1.0 / math.sqrt(HD)
    perm = np.concatenate([np.arange(0, HD, 2), np.arange(1, HD, 2)])  # even,odd
    in_maps = []
    for c in range(NC_CORES):
        heads = [HPC * c + i for i in range(HPC)]
        qcols = np.concatenate([h * HD + perm for h in heads])
        vcols = np.concatenate([np.arange(h * HD, (h + 1) * HD) for h in heads])
        Wq_c = Wq[:, qcols] * scale           # fold score scale into Wq
        Wk_c = Wk[:, qcols]
        Wv_c = Wv[:, vcols]
        Wqkv_c = np.ascontiguousarray(
            np.concatenate([Wq_c, Wk_c, Wv_c], axis=1))   # [1024, 384]
        Wproj_c = np.ascontiguousarray(Wproj[vcols, :])   # [128, 1024]
        in_maps.append({
            "xT": xT, "Wqkv_c": Wqkv_c, "Wproj_c": Wproj_c,
            "TA": TA, "TB": TB, "maskW1": W1,
            "maskW2": np.ascontiguousarray(W2.transpose(1, 0, 2).reshape(KT, 4 * QB)),
            "ident": ident, "ones_row": np.ones((1, 128), np.float32),
            "ones_col": np.ones((128, 16), np.float32),
        })
    return in_maps


# ---------------------------------------------------------------- kernel build
def build_kernel(debug_outputs=False, max_phase=99):
    nc = bass.Bass("TRN2", target_bir_lowering=False, debug=False,
                   num_devices=NC_CORES)
    xT = nc.dram_tensor("xT", [C, T], F32R, kind="ExternalInput")
    Wqkv_c = nc.dram_tensor("Wqkv_c", [C, 3 * 128], F32R, kind="ExternalInput")
    Wproj_c = nc.dram_tensor("Wproj_c", [128, C], F32R, kind="ExternalInput")
    TAd = nc.dram_tensor("TA", [128, T], F32R, kind="ExternalInput")
    TBd = nc.dram_tensor("TB", [128, T], F32R, kind="ExternalInput")
    mw1d = nc.dram_tensor("maskW1", [KT, KT], F32R, kind="ExternalInput")
    mw2d = nc.dram_tensor("maskW2", [KT, 4 * QB], F32R, kind="ExternalInput")
    identd = nc.dram_tensor("ident", [128, 128], F32R, kind="ExternalInput")
    onesrd = nc.dram_tensor("ones_row", [1, 128], F32R, kind="ExternalInput")
    onescd = nc.dram_tensor("ones_col", [128, 16], F32R, kind="ExternalInput")
    out = nc.dram_tensor("partial", [T, C], F32, kind="ExternalOutput")
    dbg = {}
    if debug_outputs:
        dbg["qT"] = nc.dram_tensor("dbg_qT", [128, T], F32R, kind="ExternalOutput")
        dbg["kT"] = nc.dram_tensor("dbg_kT", [128, T], F32R, kind="ExternalOutput")
        dbg["vT"] = nc.dram_tensor("dbg_vT", [128, T], F32R, kind="ExternalOutput")
        dbg["att0"] = nc.dram_tensor("dbg_att0", [128, L], F32R, kind="ExternalOutput")

    with tile.TileContext(nc) as tc, ExitStack() as ctx:
        const = ctx.enter_context(tc.tile_pool(name="const", bufs=1))
        # constants
    # (body continues in _build_body for clarity)
        _build_body(nc, tc, ctx, const, xT, Wqkv_c, Wproj_c, TAd, TBd, mw1d, mw2d,
                    identd, onesrd, onescd, out, dbg, max_phase)
    return nc


def _build_body(nc, tc, ctx, const, xT, Wqkv_c, Wproj_c, TAd, TBd, mw1d, mw2d,
                identd, onesrd, onescd, out, dbg, max_phase=99):
    # ---------------- constants (persistent); wq + rope tables first
    wq = const.tile([128, 8, 384], F32R)
    nc.scalar.dma_start(wq[:], Wqkv_c.ap().rearrange("(o p) f -> p o f", p=128))
    TA = const.tile([128, T], F32R)
    TB = const.tile([128, T], F32R)
    nc.scalar.dma_start(TA[:], TAd.ap())
    nc.scalar.dma_start(TB[:], TBd.ap())
    mw1 = const.tile([KT, KT], F32R)
    nc.scalar.dma_start(mw1[:], mw1d.ap())
    mw2 = const.tile([KT, 4, QB], F32R)
    nc.scalar.dma_start(mw2[:], mw2d.ap().rearrange("k (m q) -> k m q", m=4))
    ident = const.tile([128, 128], F32R)
    nc.scalar.dma_start(ident[:], identd.ap())
    ones_row = const.tile([1, 128], F32R)
    nc.scalar.dma_start(ones_row[:], onesrd.ap())
    ones_col = const.tile([128, 16], F32R)
    nc.scalar.dma_start(ones_col[:], onescd.ap())
    wp = const.tile([128, 1024], F32R)
    nc.scalar.dma_start(wp[:], Wproj_c.ap())

    qkv_sb = ctx.enter_context(tc.tile_pool(name="qkv_sb", bufs=1))
    qT = qkv_sb.tile([128, T], F32R)    # rows: h0e,h0o,h1e,h1o (roped)
    kT = qkv_sb.tile([128, T], F32R)
    vT = qkv_sb.tile([128, T], F32R)    # rows: h0 d, h1 d

    # flat pools (single allocation; phases overlap freely)
    xpool = ctx.enter_context(tc.tile_pool(name="xc", bufs=10))
    qkraw_pool = ctx.enter_context(tc.tile_pool(name="qkraw", bufs=3))
    vn_pool = ctx.enter_context(tc.tile_pool(name="vnat", bufs=1))
    att_pool = ctx.enter_context(tc.tile_pool(name="attT", bufs=15))
    avs_pool = ctx.enter_context(tc.tile_pool(name="av_sb", bufs=3))
    recip_pool = ctx.enter_context(tc.tile_pool(name="recip", bufs=2))
    bcs_pool = ctx.enter_context(tc.tile_pool(name="bcs", bufs=3))
    pj_sb = ctx.enter_context(tc.tile_pool(name="pj_sb", bufs=2))
    qkv_ps = ctx.enter_context(tc.tile_pool(name="qkv_ps", bufs=2, space="PSUM"))
    vt_ps = ctx.enter_context(tc.tile_pool(name="vt_ps", bufs=1, space="PSUM"))
    sc_ps = ctx.enter_context(tc.tile_pool(name="sc_ps", bufs=2, space="PSUM"))
    av_ps = ctx.enter_context(tc.tile_pool(name="av_ps", bufs=2, space="PSUM"))
    bc_ps = vt_ps
    pj_ps = ctx.enter_context(tc.tile_pool(name="pj_ps", bufs=1, space="PSUM"))

    vnat = [None, None]
    NCPB = L // 512                     # chunks per batch

    for b in range(B):
        vn = vn_pool.tile([128, N_KT_B, 132], F32R, tag=f"vn{b}")
        vnat[b] = vn
        # ---------- qkv + fused rope + vtrans + attention, chunk-fused
        for ncil in range(NCPB):
            nci = b * NCPB + ncil
            csl = slice(nci * 512, (nci + 1) * 512)
            xcs = []
            for k in range(8):
                xk = xpool.tile([128, 512], F32R, tag="xc")
                eng = nc.sync if k % 2 == 0 else nc.gpsimd
                eng.dma_start(
                    xk[:],
                    xT.ap().rearrange("(o p) t -> p o t", p=128)[:, k, csl])
                xcs.append(xk)
            raws = []
            for m in range(3):
                ps = qkv_ps.tile([128, 512], F32, tag="qkv")
                for k in range(8):
                    nc.tensor.matmul(ps[:], wq[:, k, m * 128:(m + 1) * 128],
                                     xcs[k][:], start=(k == 0), stop=(k == 7))
                if m == 2:
                    nc.scalar.copy(vT[:, csl], ps[:])
                else:
                    raw = qkraw_pool.tile([128, 512], F32R, tag="qkraw")
                    nc.scalar.copy(raw[:], ps[:])
                    raws.append(raw)
            # fused rope: t = raw*TA + shifted(raw)*TB
            for m, t_ in ((0, qT), (1, kT)):
                raw = raws[m]
                for g in range(4):
                    src = (g ^ 1) * 32
                    dst = g * 32
                    nc.vector.tensor_mul(t_[dst:dst + 32, csl],
                                         raw[src:src + 32, :],
                                         TB[src:src + 32, csl])
                nc.vector.tensor_mul(raw[:], raw[:], TA[:, csl])
                nc.vector.tensor_add(t_[:, csl], t_[:, csl], raw[:])

            # ---------- v transpose for this chunk's 4 k-tiles
            if max_phase < 3:
                continue
            for kt in range(ncil * 4, ncil * 4 + 4):
                ps = vt_ps.tile([128, 128], F32R, tag="vt")
                with nc.allow_low_precision(reason="f32r psum view"):
                    nc.tensor.transpose(
                        ps[:], vT[:, b * L + kt * KT: b * L + (kt + 1) * KT], ident[:])
                nc.scalar.copy(vn[:, kt, 0:64], ps[:, 0:64])
                nc.scalar.copy(vn[:, kt, 66:130], ps[:, 64:128])
            nc.scalar.dma_start(vn[:, ncil * 4:ncil * 4 + 4, 64],
                                ones_col[:, ncil * 4:ncil * 4 + 4])
            nc.scalar.dma_start(vn[:, ncil * 4:ncil * 4 + 4, 130],
                                ones_col[:, ncil * 4:ncil * 4 + 4])
            if max_phase < 4:
                continue

            # ---------- attention + proj for q block qb == ncil
            qb = ncil
            nkt = (qb + 1) * (QB // KT)     # causal k tiles
            q_sl = slice(b * L + qb * QB, b * L + (qb + 1) * QB)
            att_sb = avs_pool.tile([128, QB], F32R, tag="att_sb")
            for h in range(HPC):
                base = h * 64
                atts = []
                offs = []
                for kt in range(nkt):
                    k_sl = slice(b * L + kt * KT, b * L + (kt + 1) * KT)
                    diag = kt - qb * (QB // KT)
                    off = max(0, diag) * KT   # cols < off are fully masked
                    offs.append(off)
                    q_sl2 = slice(b * L + qb * QB + off, b * L + (qb + 1) * QB)
                    ps = sc_ps.tile([128, QB], F32, tag="sc")
                    nc.tensor.matmul(ps[:, off:], kT[base:base + 64, k_sl],
                                     qT[base:base + 64, q_sl2],
                                     start=True, stop=(diag < 0))
                    if diag >= 0:
                        nc.tensor.matmul(ps[:, off:], mw1[:], mw2[:, diag, off:],
                                         start=False, stop=True)
                    at = att_pool.tile([128, QB], F32R, tag="attT")
                    nc.scalar.activation(at[:, off:], ps[:, off:],
                                         mybir.ActivationFunctionType.Exp)
                    atts.append(at)
                if max_phase < 5:
                    continue
                aps = av_ps.tile([128, QB], F32, tag="av")
                for kt in range(nkt):
                    off = offs[kt]
                    nc.tensor.matmul(aps[0:65, off:],
                                     vnat[b][:, kt, h * 66:h * 66 + 65],
                                     atts[kt][:, off:],
                                     start=(kt == 0), stop=(kt == nkt - 1))
                rec = recip_pool.tile([1, QB], F32R, tag="rec")
                with nc.allow_low_precision(reason="softmax recip to f32r"):
                    nc.vector.reciprocal(rec[:], aps[64:65, :])
                bcp = bc_ps.tile([64, QB], F32, tag="vt")
                nc.tensor.matmul(bcp[:], ones_row[:, 0:64], rec[:],
                                 start=True, stop=True)
                avr = bcs_pool.tile([64, QB], F32R, tag="bcs")
                nc.scalar.copy(avr[:], aps[0:64, :])
                nc.vector.tensor_mul(att_sb[base:base + 64, :],
                                     avr[:], bcp[:])
            if dbg and b == 0 and qb == 0:
                nc.sync.dma_start(dbg["att0"].ap()[:, 0:QB], att_sb[:])
            if max_phase < 6:
                continue
            # proj for this q block: 4 m-tiles x 2 n-chunks
            for mt in range(QB // 128):
                row0 = qb * QB + mt * 128
                pj = pj_sb.tile([128, 1024], F32, tag="pj")
                for nn_ in range(2):
                    ps = pj_ps.tile([128, 512], F32, tag="pj_ps")
                    nc.tensor.matmul(ps[:], att_sb[:, mt * 128:(mt + 1) * 128],
                                     wp[:, nn_ * 512:(nn_ + 1) * 512],
                                     start=True, stop=True)
                    nc.vector.tensor_copy(pj[:, nn_ * 512:(nn_ + 1) * 512], ps[:])
                nc.sync.dma_start(out.ap()[b * L + row0: b * L + row0 + 128, :], pj[:])
        if dbg and b == B - 1:
            nc.sync.dma_start(dbg["qT"].ap(), qT[:])
            nc.sync.dma_start(dbg["kT"].ap(), kT[:])
            nc.sync.dma_start(dbg["vT"].ap(), vT[:])


# ---------------------------------------------------------------- reference pieces
def ref_qkvT(x, Wqkv, core):
    """Host reference for per-core qT/kT/vT (pre-rope raw for v, roped for q,k)."""
    import numpy as np
    x2 = x.reshape(T, C)
    in_maps = shard_inputs(x, Wqkv, np.zeros((C, C), np.float32))
    W = in_maps[core]["Wqkv_c"]
    qkv = x2 @ W   # [T, 384]
    return qkv[:, 0:128].T, qkv[:, 128:256].T, qkv[:, 256:384].T


# ---------------------------------------------------------------- entry point
_NC_CACHE = None
_APPLIED = False


def _ensure_patch():
    global _APPLIED
    if not _APPLIED:
        apply_tile_patch()
        _APPLIED = True


def kernel(x, Wqkv, Wproj):
    """Full-input causal attention on 8 NeuronCores (2 heads per core).

    Each core computes qkv+RoPE+causal attention for its 2 heads and a
    partial projection over its 128 channels; the host sums the 8 partial
    projections (the tensor-parallel all-reduce) and reshapes.
    """
    from concourse.bass_utils import run_bass_kernel_spmd

    global _NC_CACHE
    _ensure_patch()
    x = np.ascontiguousarray(np.asarray(x, dtype=np.float32))
    Wqkv = np.ascontiguousarray(np.asarray(Wqkv, dtype=np.float32))
    Wproj = np.ascontiguousarray(np.asarray(Wproj, dtype=np.float32))
    in_maps = shard_inputs(x, Wqkv, Wproj)
    if _NC_CACHE is None:
        nc = build_kernel(debug_outputs=False)
        split_excess_waits(nc)
        _NC_CACHE = nc
    nc = _NC_CACHE
    res = run_bass_kernel_spmd(nc, in_maps, core_ids=list(range(NC_CORES)))
    acc = np.zeros((T, C), np.float64)
    for r in res.results:
        acc += r["partial"].astype(np.float64)
    return acc.reshape(B, L, C).astype(np.float32)

